# revision 1
# baseline (speedup 1.0000x reference)
"""Causal self-attention with RoPE on 8 Trainium2 NeuronCores (Bass/Tile).

Sharding: 8 cores = 2 batch elements x 4 head-groups (4 heads each), no
collectives. Each core computes QKV for its heads from a host-pretransposed
x^T, applies RoPE, runs causally-trimmed flash-style attention, and emits a
partial output projection against its w_proj row-slice; the host sums 4
partials per batch element.

Key design points:
- All matmuls use float32r (rounded fp32): full PE rate at N>=256, measured
  ~1.5e-4 matmul relative error (vs 4x slower for exact fp32). End-to-end
  relative error vs the fp32 reference: 3.6e-4.
- Zero on-device transposes: x^T comes from the host; Q^T/K^T are computed
  d-major (2 heads packed per 128 partitions, so the K=64 S-matmuls of a
  head pair land on disjoint PE row halves); V is computed t-major.
- rotate_half is a PE matmul against a constant +-1 permutation (sign folded
  in), not a cross-partition copy.
- S^T blocks [k-tile, 512-wide q-chunk] are causally trimmed; the diagonal
  triangle mask is added in PSUM by a bf16 identity-matmul before the
  (start=False) S matmul accumulates scores.
- exp on ScalarE reads PSUM directly (scale=1/sqrt(hd) folded in); V~ has a
  ones column so the PV matmul accumulates the softmax denominator for free.
- Normalization: denominators spread over 8 partitions (tiny SBUF DMA) for
  an 8-lane reciprocal, partition_broadcast on the idle GPSIMD, one DVE
  multiply. Projection packs head pairs to full K=128 contractions.
- PSUM pool tags are shared across phases so projection overlaps the
  attention tail instead of waiting for pool release.

TimelineSim per-core kernel time: ~191 us (from 288 us for the first
correct version). Wall-clock through the axon tunnel is transfer-bound.
"""
import os

import numpy as np

import concourse.bass as bass
import concourse.mybir as mybir
import concourse.tile as tile
from concourse import bacc
from concourse.bass_utils import run_bass_kernel_spmd

# Problem shape (hardcoded per harness contract).
B, T, C, NH = 2, 2048, 1024, 16
HD = C // NH          # 64
HPC = NH // 4         # 4 heads per core
N_CORES = 8
ROPE_BASE = 10000.0
NEG = -1.0e30

F32 = mybir.dt.float32
USE_F32R = os.environ.get("KERNEL_F32R", "1") == "1"
F32R = mybir.dt.float32r if USE_F32R else mybir.dt.float32
EDT_BF16 = os.environ.get("KERNEL_EDT", "f32r") == "bf16"
XDT_BF16 = os.environ.get("KERNEL_XDT", "f32r") == "bf16"

_CACHE = {}


def _rope_tables_T():
    """cos/sin tables transposed to [HD, T], duplicated to 128 partitions
    (two 64-row head blocks), with the rotate-half sign folded into sin."""
    inv_freq = 1.0 / (ROPE_BASE ** (np.arange(0, HD, 2, dtype=np.float32) / HD))
    t = np.arange(T, dtype=np.float32)
    freqs = np.outer(t, inv_freq).astype(np.float32)      # [T, 32]
    emb = np.concatenate([freqs, freqs], axis=-1)         # [T, 64]
    cosT = np.cos(emb).T.astype(np.float32)               # [64, T]
    sinT = np.sin(emb).T.astype(np.float32)
    cos2 = np.concatenate([cosT, cosT], axis=0)           # [128, T]
    sin2 = np.concatenate([sinT, sinT], axis=0)
    return np.ascontiguousarray(cos2), np.ascontiguousarray(sin2)


def _rot_matrix():
    """rot128 so that (rot128.T @ qT) = rotate_half(q)^T per 64-row head
    block: out[d] = -in[d+32] for d<32, in[d-32] for d>=32."""
    r = np.zeros((64, 64), dtype=np.float32)
    for d in range(32):
        r[d, d + 32] = -1.0
        r[d + 32, d] = 1.0
    z = np.zeros_like(r)
    rot = np.block([[r, z], [z, r]])          # [128, 128]
    return np.ascontiguousarray(rot.T)        # lhsT layout


def _mask_tiles():
    """maskneg [128, 512] (bf16): triangle in cols 0-127, zero beyond."""
    import ml_dtypes
    k_l = np.arange(128)[:, None]
    q_l = np.arange(512)[None, :]
    m = np.where(q_l >= k_l, 0.0, NEG).astype(ml_dtypes.bfloat16)
    return m


def build_nc():
    nc = bacc.Bacc(None, target_bir_lowering=False)

    XDT = mybir.dt.bfloat16 if XDT_BF16 else F32R
    xT = nc.dram_tensor("xT", [C, T], XDT, kind="ExternalInput")
    wqk = nc.dram_tensor("wqk", [C, 8 * HD], XDT, kind="ExternalInput")
    wv = nc.dram_tensor("wv", [C, 4 * HD], XDT, kind="ExternalInput")
    wp = nc.dram_tensor("wp", [4 * HD, C], F32R, kind="ExternalInput")
    cos2_d = nc.dram_tensor("cos2", [128, T], F32, kind="ExternalInput")
    sin2_d = nc.dram_tensor("sin2", [128, T], F32, kind="ExternalInput")
    rot_d = nc.dram_tensor("rot", [128, 128], F32R, kind="ExternalInput")
    BF16 = mybir.dt.bfloat16
    maskneg_d = nc.dram_tensor("maskneg", [128, 512], BF16, kind="ExternalInput")
    ident_d = nc.dram_tensor("ident", [128, 128], BF16, kind="ExternalInput")
    outp = nc.dram_tensor("outp", [T, C], F32, kind="ExternalOutput")

    EDT = mybir.dt.bfloat16 if EDT_BF16 else F32R
    NT = T // 128    # 16 k-tiles
    NQ = T // 512    # 4 q-chunks
    max_phase = int(os.environ.get("KERNEL_PHASES", "3"))
    PROJ_MODE = os.environ.get("KERNEL_PROJ", "post")

    with tile.TileContext(nc) as tc:
        with (
            tc.tile_pool(name="persist", bufs=1) as persist,
            tc.tile_pool(name="consts", bufs=1) as consts,
            tc.tile_pool(name="psall", bufs=4, space="PSUM") as psall,
        ):
            # ---- persistent tiles (across phases) ----
            qk_packed = [
                persist.tile([128, T], F32R, name=f"qkp{w}", tag=f"qkp{w}")
                for w in range(4)
            ]
            vtil = persist.tile([128, NT, 4, HD + 1], EDT, name="vtil")
            ynorm = [
                persist.tile([128, T], F32R, name=f"ynorm{g}", tag=f"ynorm{g}")
                for g in range(2)
            ]
            BF16 = mybir.dt.bfloat16
            ident_sb = consts.tile([128, 128], BF16, name="ident_sb")
            nc.sync.dma_start(out=ident_sb, in_=ident_d[:, :])
            maskneg_sb = consts.tile([128, 512], BF16, name="maskneg_sb")
            nc.sync.dma_start(out=maskneg_sb, in_=maskneg_d[:, :])
            wp_sb = consts.tile([128, 2, C], F32R, name="wp_sb")

            # ================= Phase 1: QKV + RoPE =================
            with (
                tc.tile_pool(name="p1", bufs=1) as p1,
                tc.tile_pool(name="p1w", bufs=1) as p1w,
                tc.tile_pool(name="p1tmp", bufs=1) as p1tmp,
            ):
                xT_sb = p1.tile([128, C // 128, T], XDT, name="xT_sb")
                xT_r = xT.rearrange("(co p) t -> p co t", p=128)
                wqk_r = wqk.rearrange("(co p) n -> p co n", p=128)
                # DMA emission order = first-consumption order: wqk w=0,
                # then xT q-chunk 0, then the rest.
                wqk_sbs = []
                for w in range(4):
                    wqk_sbs.append(
                        p1w.tile([128, C // 128, 128], XDT, name=f"wqk_sb{w}",
                                 tag=f"wqk_sb{w}")
                    )
                # first matmul needs only wqk0's c0 slice + xT(q0,c0)
                nc.sync.dma_start(
                    out=wqk_sbs[0][:, 0:2, :], in_=wqk_r[:, 0:2, 0:128]
                )
                nc.sync.dma_start(
                    out=wqk_sbs[0][:, 2:8, :], in_=wqk_r[:, 2:8, 0:128]
                )
                for c in range(C // 128):
                    nc.sync.dma_start(
                        out=xT_sb[:, c, 0:512], in_=xT_r[:, c, 0:512],
                    )
                nc.sync.dma_start(
                    out=wqk_sbs[2], in_=wqk_r[:, :, 2 * 128:3 * 128]
                )
                rot_sb = p1.tile([128, 128], F32R, name="rot_sb")
                nc.sync.dma_start(out=rot_sb, in_=rot_d[:, :])
                cos2_sb = p1.tile([128, T], F32, name="cos2_sb")
                sin2_sb = p1.tile([128, T], F32, name="sin2_sb")
                # table slices interleaved with the xT chunk they unblock
                nc.sync.dma_start(out=cos2_sb[:, 0:512], in_=cos2_d[:, 0:512])
                nc.sync.dma_start(out=sin2_sb[:, 0:512], in_=sin2_d[:, 0:512])
                for q in range(1, NQ):
                    sl = slice(q * 512, (q + 1) * 512)
                    for c in range(C // 128):
                        nc.sync.dma_start(
                            out=xT_sb[:, c, sl],
                            in_=xT_r[:, c, sl],
                        )
                    nc.sync.dma_start(out=cos2_sb[:, sl], in_=cos2_d[:, sl])
                    nc.sync.dma_start(out=sin2_sb[:, sl], in_=sin2_d[:, sl])
                wv_sb = p1.tile([128, C // 128, 4 * HD], XDT, name="wv_sb")
                nc.sync.dma_start(
                    out=wv_sb, in_=wv.rearrange("(co p) n -> p co n", p=128)
                )
                for w in (1, 3):
                    nc.sync.dma_start(
                        out=wqk_sbs[w], in_=wqk_r[:, :, w * 128:(w + 1) * 128]
                    )

                # --- Q^T / K^T packed two heads per 128 partitions ---
                def do_qk(w):
                    raw = p1tmp.tile([128, T], F32R, name="raw", tag="raw",
                                     bufs=int(os.environ.get("KERNEL_RAWB", "2")))
                    for q in range(NQ):
                        sl = slice(q * 512, (q + 1) * 512)
                        pspair = psall.tile([128, 2, 512], F32,
                                            name="ps_qk", tag="psS", bufs=2)
                        ps = pspair[:, 0, :]
                        for c in range(C // 128):
                            nc.tensor.matmul(
                                ps,
                                lhsT=wqk_sbs[w][:, c, :],
                                rhs=xT_sb[:, c, sl],
                                start=(c == 0),
                                stop=(c == C // 128 - 1),
                            )
                        if os.environ.get("KERNEL_RAWC", "act") == "dve":
                            nc.vector.tensor_copy(out=raw[:, sl], in_=ps)
                        else:
                            nc.scalar.copy(out=raw[:, sl], in_=ps)
                        # rotate-half via PE permutation matrix (sign folded in)
                        psr = pspair[:, 1, :]
                        nc.tensor.matmul(psr, lhsT=rot_sb, rhs=raw[:, sl],
                                         start=True, stop=True)
                        tmp = p1tmp.tile([128, 512], F32, name="tmp",
                                         tag="tmp", bufs=2)
                        nc.vector.tensor_mul(tmp, psr, sin2_sb[:, sl])
                        cosq = p1tmp.tile([128, 512], F32, name="cosq",
                                          tag="cosq", bufs=2)
                        nc.vector.tensor_mul(cosq, raw[:, sl], cos2_sb[:, sl])
                        nc.vector.tensor_add(qk_packed[w][:, sl], cosq, tmp)

                do_qk(0)   # q heads 0,1
                do_qk(2)   # k heads 0,1 -> pair g=0 attention can start

                # --- V (t-major) + ones column ---
                ones64 = p1.tile([128, NT * 4], F32, name="ones64")
                nc.vector.memset(ones64, 1.0)
                nc.vector.tensor_copy(
                    out=vtil[:, :, :, HD:HD + 1],
                    in_=ones64.rearrange("p (a b) -> p a b", a=NT).unsqueeze(-1),
                )
                for tt in range(NT):
                    psv = psall.tile([128, 4 * HD], F32, name="psv",
                                     tag="yacc0", bufs=2,
                                     padded_shape=[128, 512])
                    for c in range(C // 128):
                        nc.tensor.matmul(
                            psv,
                            lhsT=xT_sb[:, c, tt * 128:(tt + 1) * 128],
                            rhs=wv_sb[:, c, :],
                            start=(c == 0),
                            stop=(c == C // 128 - 1),
                        )
                    nc.vector.tensor_copy(
                        out=vtil[:, tt, :, 0:HD],
                        in_=psv.rearrange("p (h d) -> p h d", h=4),
                    )

                do_qk(1)   # q heads 2,3
                do_qk(3)   # k heads 2,3

            # ================= Phase 2: attention =================
            if max_phase >= 2:
              with (
                tc.tile_pool(name="p2", bufs=1) as p2,
                tc.tile_pool(name="p2e", bufs=int(os.environ.get("KERNEL_ESBUFS", "6"))) as p2e,
                tc.tile_pool(name="p2d", bufs=int(os.environ.get("KERNEL_P2D", "2"))) as p2d,
            ):
                nc.sync.dma_start(
                    out=wp_sb, in_=wp.rearrange("(gg p) n -> p gg n", p=128)
                )
                inv_sqrt_hd = float(1.0 / np.sqrt(HD))

                def emit_proj(tt, nck):
                    pso = psall.tile([128, 512], F32, name="pso",
                                     tag=f"yacc{nck}", bufs=2,
                                     padded_shape=[128, 512])
                    for g in range(2):
                        nc.tensor.matmul(
                            pso,
                            lhsT=ynorm[g][:, tt * 128:(tt + 1) * 128],
                            rhs=wp_sb[:, g, nck * 512:(nck + 1) * 512],
                            start=(g == 0),
                            stop=(g == 1),
                        )
                    ost = p2e.tile([128, 512], F32, name="ost", tag="eS0")
                    if os.environ.get("KERNEL_OST", "dve") == "dve":
                        nc.vector.tensor_copy(out=ost, in_=pso)
                    elif nck == 0:
                        nc.scalar.copy(out=ost, in_=pso)
                    else:
                        nc.vector.tensor_copy(out=ost, in_=pso)
                    nc.sync.dma_start(
                        out=outp[tt * 128:(tt + 1) * 128,
                                 nck * 512:(nck + 1) * 512],
                        in_=ost,
                    )

                for g in range(2):          # head-pair (pack) index
                    ytils = []
                    for hh in range(2):
                        ytils.append(
                            p2.tile([HD + 1, NQ, 512], F32, name=f"ytil{g}{hh}",
                                    tag=f"ytil{hh}")
                        )

                    def make_psY():
                        return [
                            psall.tile([HD + 1, 512], F32, name=f"psY{hh}",
                                       tag=f"yacc{hh}",
                                       bufs=int(os.environ.get("KERNEL_PSYBUFS", "2")),
                                       padded_shape=[128, 512])
                            for hh in range(2)
                        ]

                    if True:
                        def emit_S(cq, j):
                            """S^T matmuls for both heads of the pair at
                            k-tile j, trimmed to the causally-valid column
                            suffix; returns the exp'd tiles + offset."""
                            # within this q-chunk, columns q_local < off are
                            # entirely below the diagonal of k-tile j
                            off = max(0, (j - 4 * cq) * 128)
                            F = 512 - off
                            qlo = cq * 512 + off
                            # both heads of the pair share one 2-bank psum
                            # tile so a single ACT op exps both (halves the
                            # fixed per-op ACT overhead)
                            psS = psall.tile(
                                [128, 2, 512], F32, name="psS", tag="psS",
                                bufs=int(os.environ.get("KERNEL_PSSBUFS", "2")),
                            )
                            for hh in range(2):
                                poff = 64 * hh
                                first = True
                                if j >= 4 * cq:
                                    # triangle mask (zeros beyond col 128)
                                    nc.tensor.matmul(
                                        psS[:, hh, 0:F],
                                        lhsT=ident_sb,
                                        rhs=maskneg_sb[:, 0:F],
                                        start=True,
                                        stop=False,
                                    )
                                    first = False
                                nc.tensor.matmul(
                                    psS[:, hh, 0:F],
                                    lhsT=qk_packed[2 + g][
                                        poff:poff + 64, j * 128:(j + 1) * 128],
                                    rhs=qk_packed[g][
                                        poff:poff + 64, qlo:qlo + F],
                                    start=first,
                                    stop=True,
                                )
                            eSp = p2e.tile([128, 2, 512], EDT, name="eSp",
                                           tag="eSp")
                            nc.scalar.activation(
                                out=eSp[:, :, 0:F], in_=psS[:, :, 0:F],
                                func=mybir.ActivationFunctionType.Exp,
                                scale=inv_sqrt_hd,
                            )
                            es = [eSp[:, 0, :], eSp[:, 1, :]]
                            return es, off

                        def emit_PV(cq, j, es_off, psY):
                            es, off = es_off
                            njt = 4 * cq + 4
                            F = 512 - off
                            for hh in range(2):
                                h = 2 * g + hh      # local head in 0..3
                                nc.tensor.matmul(
                                    psY[hh][:, off:512],
                                    lhsT=vtil[:, j, h, :],
                                    rhs=es[hh][:, 0:F],
                                    start=(j == 0),
                                    stop=(j == njt - 1),
                                )

                        def finish_chunk(cq, psY):
                          for hh in range(2):
                            # numerators + denominator row -> SBUF
                            nc.vector.tensor_copy(
                                out=ytils[hh][:, cq, :],
                                in_=psY[hh][:, :],
                            )
                            # per-chunk normalization: 1/denom (fast approx)
                            # -> partition-broadcast on the idle GPSIMD
                            # -> multiply numerators.
                            # spread the 512 denominators over 8 partitions
                            # so the iterative-divide reciprocal runs 8 lanes
                            # wide (free dim 64 instead of 512)
                            den_sp = p2d.tile([8, 64], F32, name="den_sp",
                                              tag="den_sp")
                            nc.sync.dma_start(
                                out=den_sp,
                                in_=ytils[hh][HD:HD + 1, cq, :],
                            )
                            rec_sp = p2d.tile([8, 64], F32, name="rec_sp",
                                              tag="rec_sp")
                            nc.vector.reciprocal(rec_sp, den_sp)
                            rec = p2d.tile([1, 512], F32, name="rec",
                                           tag="rec")
                            nc.sync.dma_start(out=rec, in_=rec_sp)
                            bc64 = p2d.tile([64, 512], F32, name="bc64",
                                            tag="bc64")
                            nc.gpsimd.partition_broadcast(bc64, rec)
                            if hh == 0:
                                nc.vector.tensor_mul(
                                    ynorm[g][0:64, cq * 512:(cq + 1) * 512],
                                    ytils[hh][0:64, cq, :],
                                    bc64,
                                )
                            else:
                                fix = p2d.tile([64, 512], F32R, name="fix",
                                               tag="fix")
                                nc.vector.tensor_mul(
                                    fix,
                                    ytils[hh][0:64, cq, :],
                                    bc64,
                                )
                                nc.sync.dma_start(
                                    out=ynorm[g][64:128,
                                                 cq * 512:(cq + 1) * 512],
                                    in_=fix,
                                )

                        # flattened (cq, j) stream: the S->exp->PV pipeline
                        # carries across chunk boundaries so it never drains
                        depth = int(os.environ.get("KERNEL_PIPE", "2"))
                        steps = [(cq, j) for cq in range(NQ)
                                 for j in range(4 * cq + 4)]
                        psYs = {}
                        pend = []

                        def pop_one():
                            (pcq, pj), es = pend.pop(0)
                            emit_PV(pcq, pj, es, psYs[pcq])
                            if pj == 4 * pcq + 3:       # last k-tile of chunk
                                finish_chunk(pcq, psYs.pop(pcq))

                        for (cq, j) in steps:
                            if cq not in psYs:
                                psYs[cq] = make_psY()
                            pend.append(((cq, j), emit_S(cq, j)))
                            if len(pend) > depth:
                                pop_one()
                        while pend:
                            pop_one()
                        if max_phase >= 3 and PROJ_MODE == "inter" and g == 1:
                            for tt in range(4 * cq, 4 * cq + 4):
                                for nck in range(2):
                                    emit_proj(tt, nck)

                # ============== Phase 3: projection ==============
                if max_phase >= 3 and PROJ_MODE == "post":
                    for tt in range(NT):
                        for nck in range(2):
                            emit_proj(tt, nck)

    nc.finalize()
    return nc


def _prep_in_maps(x, w_attn, w_proj):
    import ml_dtypes
    xdt = ml_dtypes.bfloat16 if XDT_BF16 else np.float32
    x = np.asarray(x, dtype=np.float32)
    w_attn = np.asarray(w_attn, dtype=np.float32)
    w_proj = np.asarray(w_proj, dtype=np.float32)

    cos2, sin2 = _rope_tables_T()
    rot = _rot_matrix()
    import ml_dtypes
    maskneg = _mask_tiles()
    ident = np.eye(128, dtype=ml_dtypes.bfloat16)

    xTs = [np.ascontiguousarray(x[b].T) for b in range(B)]
    in_maps = []
    for core in range(N_CORES):
        b = core // 4
        hbase = (core % 4) * HPC
        # wqk columns: [q_h0|q_h1, q_h2|q_h3, k_h0|k_h1, k_h2|k_h3]
        qcols = w_attn[:, hbase * HD:(hbase + HPC) * HD]
        kcols = w_attn[:, C + hbase * HD:C + (hbase + HPC) * HD]
        vcols = w_attn[:, 2 * C + hbase * HD:2 * C + (hbase + HPC) * HD]
        wqk = np.ascontiguousarray(np.concatenate([qcols, kcols], axis=1))
        wv = np.ascontiguousarray(vcols)
        wp = np.ascontiguousarray(w_proj[hbase * HD:(hbase + HPC) * HD, :])
        in_maps.append({
            "xT": xTs[b].astype(xdt) if XDT_BF16 else xTs[b],
            "wqk": wqk.astype(xdt) if XDT_BF16 else wqk,
            "wv": wv.astype(xdt) if XDT_BF16 else wv,
            "wp": wp,
            "cos2": cos2,
            "sin2": sin2,
            "rot": rot,
            "maskneg": maskneg,
            "ident": ident,
        })
    return in_maps


def _get_runner():
    """Build the SPMD jitted callable once and cache it (mirrors
    bass2jax.run_bass_via_pjrt, but reusable across kernel() calls)."""
    if "runner" in _CACHE:
        return _CACHE["runner"]

    import jax
    from jax.sharding import Mesh, PartitionSpec
    try:
        from jax.experimental.shard_map import shard_map
    except ImportError:
        from jax.shard_map import shard_map  # newer jax
    import concourse.mybir as _mybir
    from concourse import bass2jax

    nc = build_nc()
    _CACHE["nc"] = nc
    bass2jax.install_neuronx_cc_hook()

    partition_name = (
        nc.partition_id_tensor.name if nc.partition_id_tensor else None
    )
    in_names, out_names, out_avals, zero_outs = [], [], [], []
    for alloc in nc.m.functions[0].allocations:
        if not isinstance(alloc, _mybir.MemoryLocationSet):
            continue
        name = alloc.memorylocations[0].name
        if alloc.kind == "ExternalInput":
            if name != partition_name:
                in_names.append(name)
        elif alloc.kind == "ExternalOutput":
            shape = tuple(alloc.tensor_shape)
            dtype = _mybir.dt.np(alloc.dtype)
            out_names.append(name)
            out_avals.append(jax.core.ShapedArray(shape, dtype))
            zero_outs.append(np.zeros(shape, dtype))
    n_params = len(in_names)
    all_names = list(in_names) + list(out_names)
    if partition_name is not None:
        all_names.append(partition_name)
    donate = tuple(range(n_params, n_params + len(out_names)))

    def _body(*args):
        operands = list(args)
        if partition_name is not None:
            operands.append(bass2jax.partition_id_tensor())
        outs = bass2jax._bass_exec_p.bind(
            *operands,
            out_avals=tuple(out_avals),
            in_names=tuple(all_names),
            out_names=tuple(out_names),
            lowering_input_output_aliases=(),
            sim_require_finite=True,
            sim_require_nnan=True,
            nc=nc,
        )
        return tuple(outs)

    devices = jax.devices()[:N_CORES]
    mesh = Mesh(np.asarray(devices), ("core",))
    in_specs = (PartitionSpec("core"),) * (n_params + len(out_names))
    out_specs = (PartitionSpec("core"),) * len(out_names)
    sharded = jax.jit(
        shard_map(_body, mesh=mesh, in_specs=in_specs, out_specs=out_specs,
                  check_rep=False),
        donate_argnums=donate,
        keep_unused=True,
    )

    def run(in_maps):
        concat_in = [
            np.concatenate([np.asarray(in_maps[c][nm]) for c in range(N_CORES)],
                           axis=0)
            for nm in in_names
        ]
        concat_zeros = [
            np.zeros((N_CORES * z.shape[0], *z.shape[1:]), z.dtype)
            for z in zero_outs
        ]
        out_arrs = sharded(*concat_in, *concat_zeros)
        return [
            {
                nm: np.asarray(out_arrs[i]).reshape(
                    N_CORES, *out_avals[i].shape)[c]
                for i, nm in enumerate(out_names)
            }
            for c in range(N_CORES)
        ]

    _CACHE["runner"] = run
    return run


def kernel(x, w_attn, w_proj, n_head):
    assert int(n_head) == NH
    x = np.asarray(x, dtype=np.float32)
    assert x.shape == (B, T, C), x.shape

    in_maps = _prep_in_maps(x, np.asarray(w_attn), np.asarray(w_proj))
    if _CACHE.get("use_fallback"):
        results = _run_fallback(in_maps)
    else:
        try:
            run = _get_runner()
            results = run(in_maps)
        except Exception:
            _CACHE["use_fallback"] = True
            results = _run_fallback(in_maps)
    out = np.zeros((B, T, C), dtype=np.float32)
    for core in range(N_CORES):
        out[core // 4] += results[core]["outp"]
    return out


def _run_fallback(in_maps):
    """Native-NRT path (run_bass_kernel_spmd) for non-axon hosts."""
    if "nc" not in _CACHE:
        _CACHE["nc"] = build_nc()
    res = run_bass_kernel_spmd(_CACHE["nc"], in_maps,
                               core_ids=list(range(N_CORES)))
    return res.results


if __name__ == "__main__":
    rng = np.random.default_rng(0)
    x = rng.standard_normal((B, T, C)).astype(np.float32)
    wa = (rng.standard_normal((C, 3 * C)) / np.sqrt(C)).astype(np.float32)
    wpj = (rng.standard_normal((C, C)) / np.sqrt(C)).astype(np.float32)
    y = kernel(x, wa, wpj, NH)
    print("kernel ran, out:", y.shape, y.dtype, float(np.abs(y).mean()))



# revision 2
# speedup vs baseline: 1.0095x; 1.0095x over previous
"""Causal self-attention with RoPE on 8 Trainium2 NeuronCores (Bass/Tile).

Sharding: 8 cores = 2 batch elements x 4 head-groups (4 heads each), no
collectives. Each core computes QKV for its heads from a host-pretransposed
x^T, applies RoPE, runs causally-trimmed flash-style attention, and emits a
partial output projection against its w_proj row-slice; the host sums 4
partials per batch element.

Key design points:
- All matmuls use float32r (rounded fp32): full PE rate at N>=256, measured
  ~1.5e-4 matmul relative error (vs 4x slower for exact fp32). End-to-end
  relative error vs the fp32 reference: 3.6e-4.
- Zero on-device transposes: x^T comes from the host; Q^T/K^T are computed
  d-major (2 heads packed per 128 partitions, so the K=64 S-matmuls of a
  head pair land on disjoint PE row halves); V is computed t-major.
- rotate_half is a PE matmul against a constant +-1 permutation (sign folded
  in), not a cross-partition copy.
- S^T blocks [k-tile, 512-wide q-chunk] are causally trimmed; the diagonal
  triangle mask is added in PSUM by a bf16 identity-matmul before the
  (start=False) S matmul accumulates scores.
- exp on ScalarE reads PSUM directly (scale=1/sqrt(hd) folded in); V~ has a
  ones column so the PV matmul accumulates the softmax denominator for free.
- Normalization: denominators spread over 8 partitions (tiny SBUF DMA) for
  an 8-lane reciprocal, partition_broadcast on the idle GPSIMD, one DVE
  multiply. Projection packs head pairs to full K=128 contractions.
- PSUM pool tags are shared across phases so projection overlaps the
  attention tail instead of waiting for pool release.

TimelineSim per-core kernel time: ~191 us (from 288 us for the first
correct version). Wall-clock through the axon tunnel is transfer-bound.
"""
import os

import numpy as np

import concourse.bass as bass
import concourse.mybir as mybir
import concourse.tile as tile
from concourse import bacc
from concourse.bass_utils import run_bass_kernel_spmd

# Problem shape (hardcoded per harness contract).
B, T, C, NH = 2, 2048, 1024, 16
HD = C // NH          # 64
HPC = NH // 4         # 4 heads per core
N_CORES = 8
ROPE_BASE = 10000.0
NEG = -1.0e30

F32 = mybir.dt.float32
USE_F32R = os.environ.get("KERNEL_F32R", "1") == "1"
F32R = mybir.dt.float32r if USE_F32R else mybir.dt.float32
EDT_BF16 = os.environ.get("KERNEL_EDT", "f32r") == "bf16"
XDT_BF16 = os.environ.get("KERNEL_XDT", "f32r") == "bf16"

_CACHE = {}


def _rope_tables_T():
    """cos/sin tables transposed to [HD, T], duplicated to 128 partitions
    (two 64-row head blocks), with the rotate-half sign folded into sin."""
    inv_freq = 1.0 / (ROPE_BASE ** (np.arange(0, HD, 2, dtype=np.float32) / HD))
    t = np.arange(T, dtype=np.float32)
    freqs = np.outer(t, inv_freq).astype(np.float32)      # [T, 32]
    emb = np.concatenate([freqs, freqs], axis=-1)         # [T, 64]
    cosT = np.cos(emb).T.astype(np.float32)               # [64, T]
    sinT = np.sin(emb).T.astype(np.float32)
    cos2 = np.concatenate([cosT, cosT], axis=0)           # [128, T]
    sin2 = np.concatenate([sinT, sinT], axis=0)
    return np.ascontiguousarray(cos2), np.ascontiguousarray(sin2)


def _rot_matrix():
    """rot128 so that (rot128.T @ qT) = rotate_half(q)^T per 64-row head
    block: out[d] = -in[d+32] for d<32, in[d-32] for d>=32."""
    r = np.zeros((64, 64), dtype=np.float32)
    for d in range(32):
        r[d, d + 32] = -1.0
        r[d + 32, d] = 1.0
    z = np.zeros_like(r)
    rot = np.block([[r, z], [z, r]])          # [128, 128]
    return np.ascontiguousarray(rot.T)        # lhsT layout


def _mask_tiles():
    """maskneg [128, 512] (bf16): triangle in cols 0-127, zero beyond."""
    import ml_dtypes
    k_l = np.arange(128)[:, None]
    q_l = np.arange(512)[None, :]
    m = np.where(q_l >= k_l, 0.0, NEG).astype(ml_dtypes.bfloat16)
    return m


def build_nc():
    nc = bacc.Bacc(None, target_bir_lowering=False)

    XDT = mybir.dt.bfloat16 if XDT_BF16 else F32R
    xT = nc.dram_tensor("xT", [C, T], XDT, kind="ExternalInput")
    wqk = nc.dram_tensor("wqk", [C, 8 * HD], XDT, kind="ExternalInput")
    wv = nc.dram_tensor("wv", [C, 4 * HD], XDT, kind="ExternalInput")
    wp = nc.dram_tensor("wp", [4 * HD, C], F32R, kind="ExternalInput")
    cos2_d = nc.dram_tensor("cos2", [128, T], F32, kind="ExternalInput")
    sin2_d = nc.dram_tensor("sin2", [128, T], F32, kind="ExternalInput")
    rot_d = nc.dram_tensor("rot", [128, 128], F32R, kind="ExternalInput")
    BF16 = mybir.dt.bfloat16
    maskneg_d = nc.dram_tensor("maskneg", [128, 512], BF16, kind="ExternalInput")
    ident_d = nc.dram_tensor("ident", [128, 128], BF16, kind="ExternalInput")
    outp = nc.dram_tensor("outp", [T, C], F32, kind="ExternalOutput")

    EDT = mybir.dt.bfloat16 if EDT_BF16 else F32R
    NT = T // 128    # 16 k-tiles
    NQ = T // 512    # 4 q-chunks
    max_phase = int(os.environ.get("KERNEL_PHASES", "3"))
    PROJ_MODE = os.environ.get("KERNEL_PROJ", "post")

    with tile.TileContext(nc) as tc:
        with (
            tc.tile_pool(name="persist", bufs=1) as persist,
            tc.tile_pool(name="consts", bufs=1) as consts,
            tc.tile_pool(name="psall", bufs=4, space="PSUM") as psall,
        ):
            # ---- persistent tiles (across phases) ----
            qk_packed = [
                persist.tile([128, T], F32R, name=f"qkp{w}", tag=f"qkp{w}")
                for w in range(4)
            ]
            vtil = persist.tile([128, NT, 4, HD + 1], EDT, name="vtil")
            ynorm = [
                persist.tile([128, T], F32R, name=f"ynorm{g}", tag=f"ynorm{g}")
                for g in range(2)
            ]
            BF16 = mybir.dt.bfloat16
            ident_sb = consts.tile([128, 128], BF16, name="ident_sb")
            nc.sync.dma_start(out=ident_sb, in_=ident_d[:, :])
            maskneg_sb = consts.tile([128, 512], BF16, name="maskneg_sb")
            nc.sync.dma_start(out=maskneg_sb, in_=maskneg_d[:, :])
            wp_sb = consts.tile([128, 2, C], F32R, name="wp_sb")

            # ================= Phase 1: QKV + RoPE =================
            with (
                tc.tile_pool(name="p1", bufs=1) as p1,
                tc.tile_pool(name="p1w", bufs=1) as p1w,
                tc.tile_pool(name="p1tmp", bufs=1) as p1tmp,
            ):
                xT_sb = p1.tile([128, C // 128, T], XDT, name="xT_sb")
                xT_r = xT.rearrange("(co p) t -> p co t", p=128)
                wqk_r = wqk.rearrange("(co p) n -> p co n", p=128)
                # DMA emission order = first-consumption order: wqk w=0,
                # then xT q-chunk 0, then the rest.
                wqk_sbs = []
                for w in range(4):
                    wqk_sbs.append(
                        p1w.tile([128, C // 128, 128], XDT, name=f"wqk_sb{w}",
                                 tag=f"wqk_sb{w}")
                    )
                # first matmul needs only wqk0's c0 slice + xT(q0,c0)
                nc.sync.dma_start(
                    out=wqk_sbs[0][:, 0:2, :], in_=wqk_r[:, 0:2, 0:128]
                )
                nc.sync.dma_start(
                    out=wqk_sbs[0][:, 2:8, :], in_=wqk_r[:, 2:8, 0:128]
                )
                for c in range(C // 128):
                    nc.sync.dma_start(
                        out=xT_sb[:, c, 0:512], in_=xT_r[:, c, 0:512],
                    )
                nc.sync.dma_start(
                    out=wqk_sbs[2], in_=wqk_r[:, :, 2 * 128:3 * 128]
                )
                rot_sb = p1.tile([128, 128], F32R, name="rot_sb")
                nc.sync.dma_start(out=rot_sb, in_=rot_d[:, :])
                cos2_sb = p1.tile([128, T], F32, name="cos2_sb")
                sin2_sb = p1.tile([128, T], F32, name="sin2_sb")
                # table slices interleaved with the xT chunk they unblock
                nc.sync.dma_start(out=cos2_sb[:, 0:512], in_=cos2_d[:, 0:512])
                nc.sync.dma_start(out=sin2_sb[:, 0:512], in_=sin2_d[:, 0:512])
                for q in range(1, NQ):
                    sl = slice(q * 512, (q + 1) * 512)
                    for c in range(C // 128):
                        nc.sync.dma_start(
                            out=xT_sb[:, c, sl],
                            in_=xT_r[:, c, sl],
                        )
                    nc.sync.dma_start(out=cos2_sb[:, sl], in_=cos2_d[:, sl])
                    nc.sync.dma_start(out=sin2_sb[:, sl], in_=sin2_d[:, sl])
                wv_sb = p1.tile([128, C // 128, 4 * HD], XDT, name="wv_sb")
                nc.sync.dma_start(
                    out=wv_sb, in_=wv.rearrange("(co p) n -> p co n", p=128)
                )
                for w in (1, 3):
                    nc.sync.dma_start(
                        out=wqk_sbs[w], in_=wqk_r[:, :, w * 128:(w + 1) * 128]
                    )

                # --- Q^T / K^T packed two heads per 128 partitions ---
                def do_qk(w):
                    raw = p1tmp.tile([128, T], F32R, name="raw", tag="raw",
                                     bufs=int(os.environ.get("KERNEL_RAWB", "2")))
                    for q in range(NQ):
                        sl = slice(q * 512, (q + 1) * 512)
                        pspair = psall.tile([128, 2, 512], F32,
                                            name="ps_qk", tag="psS", bufs=2)
                        ps = pspair[:, 0, :]
                        for c in range(C // 128):
                            nc.tensor.matmul(
                                ps,
                                lhsT=wqk_sbs[w][:, c, :],
                                rhs=xT_sb[:, c, sl],
                                start=(c == 0),
                                stop=(c == C // 128 - 1),
                            )
                        if os.environ.get("KERNEL_RAWC", "act") == "dve":
                            nc.vector.tensor_copy(out=raw[:, sl], in_=ps)
                        else:
                            nc.scalar.copy(out=raw[:, sl], in_=ps)
                        # rotate-half via PE permutation matrix (sign folded in)
                        psr = pspair[:, 1, :]
                        nc.tensor.matmul(psr, lhsT=rot_sb, rhs=raw[:, sl],
                                         start=True, stop=True)
                        tmp = p1tmp.tile([128, 512], F32, name="tmp",
                                         tag="tmp", bufs=2)
                        nc.vector.tensor_mul(tmp, psr, sin2_sb[:, sl])
                        cosq = p1tmp.tile([128, 512], F32, name="cosq",
                                          tag="cosq", bufs=2)
                        nc.vector.tensor_mul(cosq, raw[:, sl], cos2_sb[:, sl])
                        nc.vector.tensor_add(qk_packed[w][:, sl], cosq, tmp)

                do_qk(0)   # q heads 0,1
                do_qk(2)   # k heads 0,1 -> pair g=0 attention can start

                # --- V (t-major) + ones column ---
                ones64 = p1.tile([128, NT * 4], F32, name="ones64")
                nc.vector.memset(ones64, 1.0)
                nc.vector.tensor_copy(
                    out=vtil[:, :, :, HD:HD + 1],
                    in_=ones64.rearrange("p (a b) -> p a b", a=NT).unsqueeze(-1),
                )
                for tt in range(NT):
                    psv = psall.tile([128, 4 * HD], F32, name="psv",
                                     tag="yacc0", bufs=2,
                                     padded_shape=[128, 512])
                    for c in range(C // 128):
                        nc.tensor.matmul(
                            psv,
                            lhsT=xT_sb[:, c, tt * 128:(tt + 1) * 128],
                            rhs=wv_sb[:, c, :],
                            start=(c == 0),
                            stop=(c == C // 128 - 1),
                        )
                    nc.vector.tensor_copy(
                        out=vtil[:, tt, :, 0:HD],
                        in_=psv.rearrange("p (h d) -> p h d", h=4),
                    )

                do_qk(1)   # q heads 2,3
                do_qk(3)   # k heads 2,3

            # ================= Phase 2: attention =================
            if max_phase >= 2:
              with (
                tc.tile_pool(name="p2", bufs=1) as p2,
                tc.tile_pool(name="p2e", bufs=int(os.environ.get("KERNEL_ESBUFS", "6"))) as p2e,
                tc.tile_pool(name="p2d", bufs=int(os.environ.get("KERNEL_P2D", "2"))) as p2d,
            ):
                nc.sync.dma_start(
                    out=wp_sb, in_=wp.rearrange("(gg p) n -> p gg n", p=128)
                )
                inv_sqrt_hd = float(1.0 / np.sqrt(HD))

                def emit_proj(tt, nck):
                    pso = psall.tile([128, 512], F32, name="pso",
                                     tag=f"yacc{nck}", bufs=2,
                                     padded_shape=[128, 512])
                    for g in range(2):
                        nc.tensor.matmul(
                            pso,
                            lhsT=ynorm[g][:, tt * 128:(tt + 1) * 128],
                            rhs=wp_sb[:, g, nck * 512:(nck + 1) * 512],
                            start=(g == 0),
                            stop=(g == 1),
                        )
                    ost = p2e.tile([128, 512], F32, name="ost", tag="eS0")
                    if os.environ.get("KERNEL_OST", "dve") == "dve":
                        nc.vector.tensor_copy(out=ost, in_=pso)
                    elif nck == 0:
                        nc.scalar.copy(out=ost, in_=pso)
                    else:
                        nc.vector.tensor_copy(out=ost, in_=pso)
                    nc.sync.dma_start(
                        out=outp[tt * 128:(tt + 1) * 128,
                                 nck * 512:(nck + 1) * 512],
                        in_=ost,
                    )

                for g in range(2):          # head-pair (pack) index
                    ytils = []
                    for hh in range(2):
                        ytils.append(
                            p2.tile([HD + 1, NQ, 512], F32, name=f"ytil{g}{hh}",
                                    tag=f"ytil{hh}")
                        )

                    def make_psY():
                        return [
                            psall.tile([HD + 1, 512], F32, name=f"psY{hh}",
                                       tag=f"yacc{hh}",
                                       bufs=int(os.environ.get("KERNEL_PSYBUFS", "2")),
                                       padded_shape=[128, 512])
                            for hh in range(2)
                        ]

                    if True:
                        def emit_S(cq, j):
                            """S^T matmuls for both heads of the pair at
                            k-tile j, trimmed to the causally-valid column
                            suffix; returns the exp'd tiles + offset."""
                            # within this q-chunk, columns q_local < off are
                            # entirely below the diagonal of k-tile j
                            off = max(0, (j - 4 * cq) * 128)
                            F = 512 - off
                            qlo = cq * 512 + off
                            # both heads of the pair share one 2-bank psum
                            # tile so a single ACT op exps both (halves the
                            # fixed per-op ACT overhead)
                            psS = psall.tile(
                                [128, 2, 512], F32, name="psS", tag="psS",
                                bufs=int(os.environ.get("KERNEL_PSSBUFS", "2")),
                            )
                            diag = j >= 4 * cq
                            for hh in range(2):
                                poff = 64 * hh
                                nc.tensor.matmul(
                                    psS[:, hh, 0:F],
                                    lhsT=qk_packed[2 + g][
                                        poff:poff + 64, j * 128:(j + 1) * 128],
                                    rhs=qk_packed[g][
                                        poff:poff + 64, qlo:qlo + F],
                                    start=True,
                                    stop=not diag,
                                )
                                if diag:
                                    # causal triangle only ever occupies the
                                    # first 128 columns of the trimmed block
                                    nc.tensor.matmul(
                                        psS[:, hh, 0:128],
                                        lhsT=ident_sb,
                                        rhs=maskneg_sb[:, 0:128],
                                        start=False,
                                        stop=True,
                                    )
                            eSp = p2e.tile([128, 2, 512], EDT, name="eSp",
                                           tag="eSp")
                            nc.scalar.activation(
                                out=eSp[:, :, 0:F], in_=psS[:, :, 0:F],
                                func=mybir.ActivationFunctionType.Exp,
                                scale=inv_sqrt_hd,
                            )
                            es = [eSp[:, 0, :], eSp[:, 1, :]]
                            return es, off

                        def emit_PV(cq, j, es_off, psY):
                            es, off = es_off
                            njt = 4 * cq + 4
                            F = 512 - off
                            for hh in range(2):
                                h = 2 * g + hh      # local head in 0..3
                                nc.tensor.matmul(
                                    psY[hh][:, off:512],
                                    lhsT=vtil[:, j, h, :],
                                    rhs=es[hh][:, 0:F],
                                    start=(j == 0),
                                    stop=(j == njt - 1),
                                )

                        def finish_chunk(cq, psY):
                          for hh in range(2):
                            # numerators + denominator row -> SBUF
                            nc.vector.tensor_copy(
                                out=ytils[hh][:, cq, :],
                                in_=psY[hh][:, :],
                            )
                            # per-chunk normalization: 1/denom (fast approx)
                            # -> partition-broadcast on the idle GPSIMD
                            # -> multiply numerators.
                            # spread the 512 denominators over 8 partitions
                            # so the iterative-divide reciprocal runs 8 lanes
                            # wide (free dim 64 instead of 512)
                            den_sp = p2d.tile([8, 64], F32, name="den_sp",
                                              tag="den_sp")
                            nc.sync.dma_start(
                                out=den_sp,
                                in_=ytils[hh][HD:HD + 1, cq, :],
                            )
                            rec_sp = p2d.tile([8, 64], F32, name="rec_sp",
                                              tag="rec_sp")
                            nc.vector.reciprocal(rec_sp, den_sp)
                            rec = p2d.tile([1, 512], F32, name="rec",
                                           tag="rec")
                            nc.sync.dma_start(out=rec, in_=rec_sp)
                            bc64 = p2d.tile([64, 512], F32, name="bc64",
                                            tag="bc64")
                            nc.gpsimd.partition_broadcast(bc64, rec)
                            if hh == 0:
                                nc.vector.tensor_mul(
                                    ynorm[g][0:64, cq * 512:(cq + 1) * 512],
                                    ytils[hh][0:64, cq, :],
                                    bc64,
                                )
                            else:
                                fix = p2d.tile([64, 512], F32R, name="fix",
                                               tag="fix")
                                nc.vector.tensor_mul(
                                    fix,
                                    ytils[hh][0:64, cq, :],
                                    bc64,
                                )
                                nc.sync.dma_start(
                                    out=ynorm[g][64:128,
                                                 cq * 512:(cq + 1) * 512],
                                    in_=fix,
                                )

                        # flattened (cq, j) stream: the S->exp->PV pipeline
                        # carries across chunk boundaries so it never drains
                        depth = int(os.environ.get("KERNEL_PIPE", "2"))
                        steps = [(cq, j) for cq in range(NQ)
                                 for j in range(4 * cq + 4)]
                        psYs = {}
                        pend = []

                        def pop_one():
                            (pcq, pj), es = pend.pop(0)
                            emit_PV(pcq, pj, es, psYs[pcq])
                            if pj == 4 * pcq + 3:       # last k-tile of chunk
                                finish_chunk(pcq, psYs.pop(pcq))

                        for (cq, j) in steps:
                            if cq not in psYs:
                                psYs[cq] = make_psY()
                            pend.append(((cq, j), emit_S(cq, j)))
                            if len(pend) > depth:
                                pop_one()
                        while pend:
                            pop_one()
                        if max_phase >= 3 and PROJ_MODE == "inter" and g == 1:
                            for tt in range(4 * cq, 4 * cq + 4):
                                for nck in range(2):
                                    emit_proj(tt, nck)

                # ============== Phase 3: projection ==============
                if max_phase >= 3 and PROJ_MODE == "post":
                    for tt in range(NT):
                        for nck in range(2):
                            emit_proj(tt, nck)

    nc.finalize()
    return nc


def _prep_in_maps(x, w_attn, w_proj):
    import ml_dtypes
    xdt = ml_dtypes.bfloat16 if XDT_BF16 else np.float32
    x = np.asarray(x, dtype=np.float32)
    w_attn = np.asarray(w_attn, dtype=np.float32)
    w_proj = np.asarray(w_proj, dtype=np.float32)

    cos2, sin2 = _rope_tables_T()
    rot = _rot_matrix()
    import ml_dtypes
    maskneg = _mask_tiles()
    ident = np.eye(128, dtype=ml_dtypes.bfloat16)

    xTs = [np.ascontiguousarray(x[b].T) for b in range(B)]
    in_maps = []
    for core in range(N_CORES):
        b = core // 4
        hbase = (core % 4) * HPC
        # wqk columns: [q_h0|q_h1, q_h2|q_h3, k_h0|k_h1, k_h2|k_h3]
        qcols = w_attn[:, hbase * HD:(hbase + HPC) * HD]
        kcols = w_attn[:, C + hbase * HD:C + (hbase + HPC) * HD]
        vcols = w_attn[:, 2 * C + hbase * HD:2 * C + (hbase + HPC) * HD]
        wqk = np.ascontiguousarray(np.concatenate([qcols, kcols], axis=1))
        wv = np.ascontiguousarray(vcols)
        wp = np.ascontiguousarray(w_proj[hbase * HD:(hbase + HPC) * HD, :])
        in_maps.append({
            "xT": xTs[b].astype(xdt) if XDT_BF16 else xTs[b],
            "wqk": wqk.astype(xdt) if XDT_BF16 else wqk,
            "wv": wv.astype(xdt) if XDT_BF16 else wv,
            "wp": wp,
            "cos2": cos2,
            "sin2": sin2,
            "rot": rot,
            "maskneg": maskneg,
            "ident": ident,
        })
    return in_maps


def _get_runner():
    """Build the SPMD jitted callable once and cache it (mirrors
    bass2jax.run_bass_via_pjrt, but reusable across kernel() calls)."""
    if "runner" in _CACHE:
        return _CACHE["runner"]

    import jax
    from jax.sharding import Mesh, PartitionSpec
    try:
        from jax.experimental.shard_map import shard_map
    except ImportError:
        from jax.shard_map import shard_map  # newer jax
    import concourse.mybir as _mybir
    from concourse import bass2jax

    nc = build_nc()
    _CACHE["nc"] = nc
    bass2jax.install_neuronx_cc_hook()

    partition_name = (
        nc.partition_id_tensor.name if nc.partition_id_tensor else None
    )
    in_names, out_names, out_avals, zero_outs = [], [], [], []
    for alloc in nc.m.functions[0].allocations:
        if not isinstance(alloc, _mybir.MemoryLocationSet):
            continue
        name = alloc.memorylocations[0].name
        if alloc.kind == "ExternalInput":
            if name != partition_name:
                in_names.append(name)
        elif alloc.kind == "ExternalOutput":
            shape = tuple(alloc.tensor_shape)
            dtype = _mybir.dt.np(alloc.dtype)
            out_names.append(name)
            out_avals.append(jax.core.ShapedArray(shape, dtype))
            zero_outs.append(np.zeros(shape, dtype))
    n_params = len(in_names)
    all_names = list(in_names) + list(out_names)
    if partition_name is not None:
        all_names.append(partition_name)
    donate = tuple(range(n_params, n_params + len(out_names)))

    def _body(*args):
        operands = list(args)
        if partition_name is not None:
            operands.append(bass2jax.partition_id_tensor())
        outs = bass2jax._bass_exec_p.bind(
            *operands,
            out_avals=tuple(out_avals),
            in_names=tuple(all_names),
            out_names=tuple(out_names),
            lowering_input_output_aliases=(),
            sim_require_finite=True,
            sim_require_nnan=True,
            nc=nc,
        )
        return tuple(outs)

    devices = jax.devices()[:N_CORES]
    mesh = Mesh(np.asarray(devices), ("core",))
    in_specs = (PartitionSpec("core"),) * (n_params + len(out_names))
    out_specs = (PartitionSpec("core"),) * len(out_names)
    sharded = jax.jit(
        shard_map(_body, mesh=mesh, in_specs=in_specs, out_specs=out_specs,
                  check_rep=False),
        donate_argnums=donate,
        keep_unused=True,
    )

    def run(in_maps):
        concat_in = [
            np.concatenate([np.asarray(in_maps[c][nm]) for c in range(N_CORES)],
                           axis=0)
            for nm in in_names
        ]
        concat_zeros = [
            np.zeros((N_CORES * z.shape[0], *z.shape[1:]), z.dtype)
            for z in zero_outs
        ]
        out_arrs = sharded(*concat_in, *concat_zeros)
        return [
            {
                nm: np.asarray(out_arrs[i]).reshape(
                    N_CORES, *out_avals[i].shape)[c]
                for i, nm in enumerate(out_names)
            }
            for c in range(N_CORES)
        ]

    _CACHE["runner"] = run
    return run


def kernel(x, w_attn, w_proj, n_head):
    assert int(n_head) == NH
    x = np.asarray(x, dtype=np.float32)
    assert x.shape == (B, T, C), x.shape

    in_maps = _prep_in_maps(x, np.asarray(w_attn), np.asarray(w_proj))
    if _CACHE.get("use_fallback"):
        results = _run_fallback(in_maps)
    else:
        try:
            run = _get_runner()
            results = run(in_maps)
        except Exception:
            _CACHE["use_fallback"] = True
            results = _run_fallback(in_maps)
    out = np.zeros((B, T, C), dtype=np.float32)
    for core in range(N_CORES):
        out[core // 4] += results[core]["outp"]
    return out


def _run_fallback(in_maps):
    """Native-NRT path (run_bass_kernel_spmd) for non-axon hosts."""
    if "nc" not in _CACHE:
        _CACHE["nc"] = build_nc()
    res = run_bass_kernel_spmd(_CACHE["nc"], in_maps,
                               core_ids=list(range(N_CORES)))
    return res.results


if __name__ == "__main__":
    rng = np.random.default_rng(0)
    x = rng.standard_normal((B, T, C)).astype(np.float32)
    wa = (rng.standard_normal((C, 3 * C)) / np.sqrt(C)).astype(np.float32)
    wpj = (rng.standard_normal((C, C)) / np.sqrt(C)).astype(np.float32)
    y = kernel(x, wa, wpj, NH)
    print("kernel ran, out:", y.shape, y.dtype, float(np.abs(y).mean()))

